# revision 4
# baseline (speedup 1.0000x reference)
"""Trainium2 Bass kernel for quantized ConvBlock (fake-quant -> conv3x3 -> BN -> relu6 fake-quant).

Strategy
--------
Data-parallel over batch: 32 images -> 4 per NeuronCore x 8 cores.

Math: the reference fake-quantizes activations to the 256-level grid
k*(6/255), k in [0,255], and weights to m*(s/127), m in [-127,127],
s = max|w|.  Both integer grids are exactly representable in bf16, so the
conv reduces to an *integer* matmul accumulated in fp32 PSUM — exact —
and runs at full bf16 TensorE rate.  Per (r,s) tap the 3x3 conv is a
128(Cin) x 128(Cout-half) matmul over pixels; 9 taps accumulate in PSUM.

Rounding: ACT has no rint, so round-to-nearest-even is done on DVE with
the fp32 magic-number trick (v + 1.5*2^23) - 1.5*2^23, valid for
|v| < 2^22 (values here are <= ~1e5).

Fused epilogue per PSUM tile:
  ACT : t = Relu(a2_c * conv + b2_c)     (a2 = 42.5*bn_scale*qscale, per-channel)
  DVE : u = min(t,255) + MAGIC
  DVE : out = (u - MAGIC) * (6/255)
"""

import numpy as np

import concourse.bass as bass
import concourse.mybir as mybir
import concourse.tile as tile
from concourse import bacc, bass_isa
from concourse.bass_utils import run_bass_kernel_spmd

# ---- problem constants (hardcoded per contract) ----
N, C, H, W = 32, 128, 56, 56
O = 256
NCORES = 8
NIMG = N // NCORES  # images per core
HP, WP = H + 2, W + 2  # zero-padded input plane
ROWS_PER_CHUNK = 8
NCHUNK = H // ROWS_PER_CHUNK  # 7
FREE = ROWS_PER_CHUNK * W  # 448 <= 512 (one PSUM bank)

MAGIC = 12582912.0  # 1.5 * 2**23 : fp32 RNE round-to-int trick
QA = 42.5  # 255/6
STEP = float(np.float32(6.0 / 255.0))
BN_EPS = 1e-5

f32 = mybir.dt.float32
bf16 = mybir.dt.bfloat16
ALU = mybir.AluOpType
ACTF = mybir.ActivationFunctionType


def _build_body(tc):
    nc = tc.nc
    xs = nc.dram_tensor("xs", [NIMG, C, H, W], f32, kind="ExternalInput")
    wt = nc.dram_tensor("wt", [O, C, 3, 3], f32, kind="ExternalInput")
    gm = nc.dram_tensor("gm", [O], f32, kind="ExternalInput")
    bt = nc.dram_tensor("bt", [O], f32, kind="ExternalInput")
    mn = nc.dram_tensor("mn", [O], f32, kind="ExternalInput")
    vr = nc.dram_tensor("vr", [O], f32, kind="ExternalInput")
    out = nc.dram_tensor("out", [NIMG, O, H, W], f32, kind="ExternalOutput")

    from contextlib import ExitStack

    with ExitStack() as ctx:
        const = ctx.enter_context(tc.tile_pool(name="const", bufs=1))
        wpool = ctx.enter_context(tc.tile_pool(name="wpool", bufs=1))
        xqpool = ctx.enter_context(tc.tile_pool(name="xqp", bufs=1))
        xraw = ctx.enter_context(tc.tile_pool(name="xraw", bufs=2))
        tq = ctx.enter_context(tc.tile_pool(name="tq", bufs=2))
        psum = ctx.enter_context(tc.tile_pool(name="psum", bufs=8, space="PSUM"))
        post = ctx.enter_context(tc.tile_pool(name="post", bufs=4))
        outp = ctx.enter_context(tc.tile_pool(name="outp", bufs=4))

        # ================= weights: load, quantize to integer bf16 =========
        # SBUF layout [i(part), o, rs] from DRAM (o, i, r, s)
        wf = wpool.tile([C, O, 9], f32)
        nc.sync.dma_start(wf[:], wt.ap().rearrange("o i h w -> i o (h w)"))

        wabs = const.tile([C, 1], f32)
        nc.vector.tensor_reduce(
            wabs[:], wf[:], axis=mybir.AxisListType.XY, op=ALU.max,
            apply_absolute_value=True,
        )
        smax = const.tile([C, 1], f32)
        nc.gpsimd.partition_all_reduce(
            smax[:], wabs[:], channels=C, reduce_op=bass_isa.ReduceOp.absmax
        )
        srcp = const.tile([C, 1], f32)
        nc.vector.reciprocal(srcp[:], smax[:])
        winv = const.tile([C, 1], f32)  # 127/s  (HW iterative divide)
        nc.vector.tensor_scalar(winv[:], srcp[:], 127.0, None, op0=ALU.mult)

        # wq[i, rs, o] = rint(w * 127/s) as bf16 (integers in [-127,127])
        wtmp = wpool.tile([C, 9, O], f32)
        nc.vector.tensor_scalar(
            wtmp[:], wf[:].rearrange("i o r -> i r o"), winv[:], MAGIC,
            op0=ALU.mult, op1=ALU.add,
        )
        wq = wpool.tile([C, 9, O], bf16)
        nc.vector.tensor_scalar(wq[:], wtmp[:], MAGIC, None, op0=ALU.subtract)

        # ================= BN constants ====================================
        # channel (h*128+p) -> partition p, column h
        gmt = const.tile([128, 2], f32)
        btt = const.tile([128, 2], f32)
        mnt = const.tile([128, 2], f32)
        vrt = const.tile([128, 2], f32)
        nc.sync.dma_start(gmt[:], gm.ap().rearrange("(h p) -> p h", p=128))
        nc.sync.dma_start(btt[:], bt.ap().rearrange("(h p) -> p h", p=128))
        nc.sync.dma_start(mnt[:], mn.ap().rearrange("(h p) -> p h", p=128))
        nc.sync.dma_start(vrt[:], vr.ap().rearrange("(h p) -> p h", p=128))

        # sqrt(var+eps) via ACT LUT + 2 Newton steps (LUT sqrt is low-precision)
        veps = const.tile([128, 2], f32)
        nc.vector.tensor_scalar(veps[:], vrt[:], BN_EPS, None, op0=ALU.add)
        sv = const.tile([128, 2], f32)
        nc.scalar.activation(sv[:], veps[:], ACTF.Sqrt)
        cur = sv
        for it in range(2):
            rc = const.tile([128, 2], f32, name=f"nwt_r{it}")
            nc.vector.reciprocal(rc[:], cur[:])
            d = const.tile([128, 2], f32, name=f"nwt_d{it}")
            nc.vector.tensor_tensor(d[:], veps[:], rc[:], op=ALU.mult)
            s2 = const.tile([128, 2], f32, name=f"nwt_s{it}")
            nc.vector.tensor_tensor(s2[:], cur[:], d[:], op=ALU.add)
            nxt = const.tile([128, 2], f32, name=f"nwt_n{it}")
            nc.vector.tensor_scalar(nxt[:], s2[:], 0.5, None, op0=ALU.mult)
            cur = nxt

        svr = const.tile([128, 2], f32)
        nc.vector.reciprocal(svr[:], cur[:])
        bnscale = const.tile([128, 2], f32)
        nc.vector.tensor_tensor(bnscale[:], gmt[:], svr[:], op=ALU.mult)

        # a2 = bnscale * s/127   (42.5 * 6/255 == 1)
        qs2 = const.tile([128, 1], f32)
        nc.vector.tensor_scalar(qs2[:], smax[:], 1.0 / 127.0, None, op0=ALU.mult)
        a2 = const.tile([128, 2], f32)
        nc.vector.tensor_scalar(a2[:], bnscale[:], qs2[:], None, op0=ALU.mult)
        # b2 = 42.5 * (beta - mean*bnscale)
        msc = const.tile([128, 2], f32)
        nc.vector.tensor_tensor(msc[:], mnt[:], bnscale[:], op=ALU.mult)
        bmm = const.tile([128, 2], f32)
        nc.vector.tensor_tensor(bmm[:], btt[:], msc[:], op=ALU.subtract)
        b2 = const.tile([128, 2], f32)
        nc.vector.tensor_scalar(b2[:], bmm[:], QA, None, op0=ALU.mult)

        # ================= per-image: quantize input, conv, epilogue =======
        xq_tiles = []
        for im in range(NIMG):
            xq = xqpool.tile([C, HP, WP], bf16, name=f"xq{im}")
            nc.vector.memset(xq[:], 0.0)
            xq_tiles.append(xq)

        for im in range(NIMG):
            xq = xq_tiles[im]
            xr = xraw.tile([C, H * W], f32, name="xr")
            nc.sync.dma_start(xr[:], xs.ap()[im].rearrange("c h w -> c (h w)"))
            t1 = tq.tile([C, H * W], f32, name="t1")
            nc.scalar.activation(t1[:], xr[:], ACTF.Relu, scale=QA)
            t2 = tq.tile([C, H * W], f32, name="t2")
            nc.vector.tensor_scalar(t2[:], t1[:], 255.0, MAGIC, op0=ALU.min, op1=ALU.add)
            # integer-valued bf16 into padded interior
            nc.vector.tensor_scalar(
                xq[:, 1 : H + 1, 1 : W + 1],
                t2[:].rearrange("c (h w) -> c h w", h=H),
                MAGIC, None, op0=ALU.subtract,
            )

            for ch in range(NCHUNK):
                for half in range(2):
                    ps = psum.tile([128, FREE], f32, name="ps")
                    for r in range(3):
                        for s in range(3):
                            rs = r * 3 + s
                            nc.tensor.matmul(
                                ps[:],
                                wq[:, rs, half * 128 : (half + 1) * 128],
                                xq[:, ch * ROWS_PER_CHUNK + r : ch * ROWS_PER_CHUNK + r + ROWS_PER_CHUNK, s : s + W],
                                start=(rs == 0),
                                stop=(rs == 8),
                            )
                    tpost = post.tile([128, FREE], f32, name="tpost")
                    nc.scalar.activation(
                        tpost[:], ps[:], ACTF.Relu,
                        bias=b2[:, half : half + 1], scale=a2[:, half : half + 1],
                    )
                    u = post.tile([128, FREE], f32, name="u")
                    nc.vector.tensor_scalar(
                        u[:], tpost[:], 255.0, MAGIC, op0=ALU.min, op1=ALU.add
                    )
                    ov = outp.tile([128, FREE], f32, name="ov")
                    nc.vector.tensor_scalar(
                        ov[:], u[:], MAGIC, STEP, op0=ALU.subtract, op1=ALU.mult
                    )
                    nc.sync.dma_start(
                        out.ap()[im, half * 128 : (half + 1) * 128]
                        .rearrange("o h w -> o (h w)")[:, ch * FREE : (ch + 1) * FREE],
                        ov[:],
                    )


_CACHED = None


def _get_program():
    global _CACHED
    if _CACHED is None:
        nc = bacc.Bacc(
            "TRN2", target_bir_lowering=False, debug=False, num_devices=NCORES
        )
        with tile.TileContext(nc) as tc:
            _build_body(tc)
        nc.compile()
        _CACHED = nc
    return _CACHED


def run_on_cores(inputs, trace=False, **kw):
    """Run the SPMD kernel; returns (full_output, BassKernelResults)."""
    nc = _get_program()
    x = np.ascontiguousarray(inputs["x"], dtype=np.float32)
    in_maps = []
    for c in range(NCORES):
        in_maps.append(
            {
                "xs": np.ascontiguousarray(x[c * NIMG : (c + 1) * NIMG]),
                "wt": np.ascontiguousarray(inputs["weight"], dtype=np.float32),
                "gm": np.ascontiguousarray(inputs["gamma"], dtype=np.float32),
                "bt": np.ascontiguousarray(inputs["beta"], dtype=np.float32),
                "mn": np.ascontiguousarray(inputs["mean"], dtype=np.float32),
                "vr": np.ascontiguousarray(inputs["var"], dtype=np.float32),
            }
        )
    res = run_bass_kernel_spmd(nc, in_maps, list(range(NCORES)), trace=trace, **kw)
    full = np.concatenate([res.results[c]["out"] for c in range(NCORES)], axis=0)
    return full.astype(np.float32), res


def kernel(**inputs) -> np.ndarray:
    full, _ = run_on_cores(inputs)
    return full


# revision 31
# speedup vs baseline: 57690.8310x; 57690.8310x over previous
"""Trainium2 Bass kernel for quantized ConvBlock (fake-quant -> conv3x3 -> BN -> relu6 fake-quant).

Strategy
--------
Data-parallel over batch: 32 images -> 4 per NeuronCore x 8 cores.

Math: the reference fake-quantizes activations to the 256-level grid
k*(6/255), k in [0,255], and weights to m*(s/127), m in [-127,127],
s = max|w|.  Both integer grids are exactly representable in bf16, so the
conv reduces to an *integer* matmul accumulated in fp32 PSUM — exact —
and runs at full bf16 TensorE rate.  Per (r,s) tap the 3x3 conv is a
128(Cin) x 128(Cout-half) matmul over pixels; 9 taps accumulate in PSUM.

Rounding: ACT has no rint, so round-to-nearest-even is done on DVE with
the fp32 magic-number trick (v + 1.5*2^23) - 1.5*2^23, valid for
|v| < 2^22 (values here are <= ~1e5).

Fused epilogue per PSUM tile:
  ACT : t = Relu(a2_c * conv + b2_c)     (a2 = 42.5*bn_scale*qscale, per-channel)
  DVE : u = min(t,255) + MAGIC
  DVE : out = (u - MAGIC) * (6/255)
"""

import numpy as np

import concourse.bass as bass
import concourse.mybir as mybir
import concourse.tile as tile
from concourse import bacc, bass_isa
from concourse.bass_utils import run_bass_kernel_spmd

# ---- problem constants (hardcoded per contract) ----
N, C, H, W = 32, 128, 56, 56
O = 256
NCORES = 8
NIMG = N // NCORES  # images per core
HP, WP = H + 2, W + 2  # zero-padded input plane
ROWS_PER_CHUNK = 8
NCHUNK = H // ROWS_PER_CHUNK  # 7
FREE = ROWS_PER_CHUNK * W  # 448 <= 512 (one PSUM bank)

MAGIC = 12582912.0  # 1.5 * 2**23 : fp32 RNE round-to-int trick
QA = 42.5  # 255/6
STEP = float(np.float32(6.0 / 255.0))
BN_EPS = 1e-5

f32 = mybir.dt.float32
bf16 = mybir.dt.bfloat16
ALU = mybir.AluOpType
ACTF = mybir.ActivationFunctionType


def _build_body(tc):
    nc = tc.nc
    xs = nc.dram_tensor("xs", [NIMG, C, H, W], f32, kind="ExternalInput")
    wt = nc.dram_tensor("wt", [O, C, 3, 3], f32, kind="ExternalInput")
    gm = nc.dram_tensor("gm", [O], f32, kind="ExternalInput")
    bt = nc.dram_tensor("bt", [O], f32, kind="ExternalInput")
    mn = nc.dram_tensor("mn", [O], f32, kind="ExternalInput")
    vr = nc.dram_tensor("vr", [O], f32, kind="ExternalInput")
    out = nc.dram_tensor("out", [NIMG, O, H, W], f32, kind="ExternalOutput")

    from contextlib import ExitStack

    with ExitStack() as ctx:
        const = ctx.enter_context(tc.tile_pool(name="const", bufs=1))
        wpool = ctx.enter_context(tc.tile_pool(name="wpool", bufs=1))
        xqpool = ctx.enter_context(tc.tile_pool(name="xqp", bufs=1))
        xraw = ctx.enter_context(tc.tile_pool(name="xraw", bufs=2))
        tq = ctx.enter_context(tc.tile_pool(name="tq", bufs=2))
        psum = ctx.enter_context(tc.tile_pool(name="psum", bufs=8, space="PSUM"))
        post = ctx.enter_context(tc.tile_pool(name="post", bufs=4))
        outp = ctx.enter_context(tc.tile_pool(name="outp", bufs=4))

        from concourse.masks import make_identity

        ident = const.tile([128, 128], bf16)
        make_identity(nc, ident[:])

        # ================= weights: load, quantize to integer bf16 =========
        # Natural-layout contiguous load [o(part), i*9] (2 halves), quantize
        # there, then 18 PE transposes (bf16) into stationary layout
        # [i(part), rs, o] per half.  Avoids the 36B-descriptor strided gather.
        wt_nat = wt.ap().rearrange("o i h w -> o (i h w)")
        wnat = []
        for h in range(2):
            wn = wpool.tile([128, C * 9], f32, name=f"wnat{h}")
            # one half per HWDGE ring (SP / ACT): parallel triggers on silicon
            eng = nc.sync if h == 0 else nc.scalar
            eng.dma_start(wn[:], wt_nat[h * 128 : (h + 1) * 128, :])
            wnat.append(wn)

        # ========== BN constants, part 1 (independent of weight scale) ======
        # Emitted first so the single ACT Sqrt triggers the (only) activation
        # table-set load at t~0; Relu is a filler fn present in every set.
        # channel (h*128+p) -> partition p, column h
        gmt = const.tile([128, 2], f32)
        btt = const.tile([128, 2], f32)
        mnt = const.tile([128, 2], f32)
        vrt = const.tile([128, 2], f32)
        nc.sync.dma_start(vrt[:], vr.ap().rearrange("(h p) -> p h", p=128))
        nc.sync.dma_start(gmt[:], gm.ap().rearrange("(h p) -> p h", p=128))
        nc.sync.dma_start(btt[:], bt.ap().rearrange("(h p) -> p h", p=128))
        nc.sync.dma_start(mnt[:], mn.ap().rearrange("(h p) -> p h", p=128))

        # bnscale = gamma * rsqrt(var+eps): ACT Sqrt seed + approx-recip +
        # 2 Newton rsqrt refinements (all cheap DVE ops, no iterative divide)
        veps = const.tile([128, 2], f32)
        nc.vector.tensor_scalar(veps[:], vrt[:], BN_EPS, None, op0=ALU.add)
        sv = const.tile([128, 2], f32)
        nc.scalar.activation(sv[:], veps[:], ACTF.Sqrt)
        r_scr = const.tile([128, 2], f32)
        r_cur = const.tile([128, 2], f32)
        nc.vector.reciprocal_approx_accurate(r_cur[:], sv[:], r_scr[:])
        cur = r_cur
        for it in range(2):
            t_sq = const.tile([128, 2], f32, name=f"rs_t{it}")
            nc.vector.tensor_tensor(t_sq[:], cur[:], cur[:], op=ALU.mult)
            t_u = const.tile([128, 2], f32, name=f"rs_u{it}")
            nc.vector.tensor_tensor(t_u[:], veps[:], t_sq[:], op=ALU.mult)
            t_c = const.tile([128, 2], f32, name=f"rs_c{it}")
            nc.vector.tensor_scalar(t_c[:], t_u[:], -0.5, 1.5, op0=ALU.mult, op1=ALU.add)
            t_n = const.tile([128, 2], f32, name=f"rs_n{it}")
            nc.vector.tensor_tensor(t_n[:], cur[:], t_c[:], op=ALU.mult)
            cur = t_n
        bnscale = const.tile([128, 2], f32)
        nc.vector.tensor_tensor(bnscale[:], gmt[:], cur[:], op=ALU.mult)
        # b2 = 42.5 * (beta - mean*bnscale)
        msc = const.tile([128, 2], f32)
        nc.vector.tensor_tensor(msc[:], mnt[:], bnscale[:], op=ALU.mult)
        bmm = const.tile([128, 2], f32)
        nc.vector.tensor_tensor(bmm[:], btt[:], msc[:], op=ALU.subtract)
        b2 = const.tile([128, 2], f32)
        nc.vector.tensor_scalar(b2[:], bmm[:], QA, None, op0=ALU.mult)


        # x image 0 load right after weights: its quant chain overlaps weight prep
        xr_tiles = {}
        xr0 = xraw.tile([C, H * W], f32, name="xr")
        nc.sync.dma_start(xr0[:], xs.ap()[0].rearrange("c h w -> c (h w)"))
        xr_tiles[0] = xr0


        habs = []
        for h in range(2):
            t = const.tile([128, 1], f32, name=f"wabs{h}")
            nc.vector.tensor_reduce(
                t[:], wnat[h][:], axis=mybir.AxisListType.X, op=ALU.max,
                apply_absolute_value=True,
            )
            habs.append(t)
        wabs = const.tile([128, 1], f32)
        nc.vector.tensor_tensor(wabs[:], habs[0][:], habs[1][:], op=ALU.max)
        smax = const.tile([C, 1], f32)
        nc.gpsimd.partition_all_reduce(
            smax[:], wabs[:], channels=C, reduce_op=bass_isa.ReduceOp.absmax
        )
        # 1/s via approx reciprocal + extra Newton pass (sub-ULP, ~2x faster
        # than the iterative-divide reciprocal which sits on the critical path)
        from concourse.dve_ops import RECIPROCAL_APPROX_NR

        rscr = const.tile([C, 1], f32)
        srcp = const.tile([C, 1], f32)
        nc.vector.reciprocal_approx_accurate(srcp[:], smax[:], rscr[:])
        srcp2 = const.tile([C, 1], f32)
        nc.vector._custom_dve(
            RECIPROCAL_APPROX_NR, out=srcp2[:], in0=smax[:], in1=srcp[:], s0=2.0
        )
        winv = const.tile([C, 1], f32)  # 127/s
        nc.vector.tensor_scalar(winv[:], srcp2[:], 127.0, None, op0=ALU.mult)

        # quantize in natural layout: integers in [-127,127], bf16
        wqn = []
        for h in range(2):
            wtmp = wpool.tile([128, C * 9], f32, name=f"wtmp{h}")
            nc.vector.tensor_scalar(
                wtmp[:], wnat[h][:], winv[:], MAGIC, op0=ALU.mult, op1=ALU.add
            )
            wq_h = wpool.tile([128, C * 9], bf16, name=f"wqn{h}")
            nc.vector.tensor_scalar(wq_h[:], wtmp[:], MAGIC, None, op0=ALU.subtract)
            wqn.append(wq_h)

        # a2 = bnscale * s/127   (42.5 * 6/255 == 1)
        qs2 = const.tile([128, 1], f32)
        nc.vector.tensor_scalar(qs2[:], smax[:], 1.0 / 127.0, None, op0=ALU.mult)
        a2 = const.tile([128, 2], f32)
        nc.vector.tensor_scalar(a2[:], bnscale[:], qs2[:], None, op0=ALU.mult)

        # transpose [o, i] -> [i, o] per (half, rs) via PE; per-half wq tiles
        # so half-0 matmuls unblock before half-1 transposes finish
        wqh = []
        for h in range(2):
            wq_t = wpool.tile([C, 9, 128], bf16, name=f"wqT{h}")
            wqn_r = wqn[h][:].rearrange("o (i r) -> o r i", r=9)
            for rs in range(9):
                pst = psum.tile([128, 128], bf16, name="pst", bufs=2)
                nc.tensor.transpose(pst[:], wqn_r[:, rs, :], ident[:])
                if rs % 2 == 0:
                    nc.scalar.copy(wq_t[:, rs, :], pst[:])
                else:
                    nc.vector.tensor_copy(wq_t[:, rs, :], pst[:])
            wqh.append(wq_t)

        # ================= per-image: quantize input, conv, epilogue =======
        # padded input plane split into two row-bands with a 2-row overlap so
        # the first chunks' matmuls start before the whole image is quantized:
        #   band A = padded rows 0..33  (chunks 0..3), writes orig rows 0..32
        #   band B = padded rows 32..57 (chunks 4..6), writes orig rows 31..55
        CHA = 4  # chunks in band A
        RA = CHA * ROWS_PER_CHUNK + 2  # 34 padded rows
        POB = (CHA * ROWS_PER_CHUNK) - 2  # band B starts at padded row 32
        RB = HP - POB  # 26 padded rows

        for im in range(NIMG):
            xqa = xqpool.tile([C, RA, WP], bf16, name=f"xqa{im}")
            xqb = xqpool.tile([C, RB, WP], bf16, name=f"xqb{im}")
            # zero only the pad ring; interior is overwritten by the quant pass
            nc.gpsimd.memset(xqa[:, 0, :], 0.0)
            nc.gpsimd.memset(xqa[:, 1:RA, 0], 0.0)
            nc.gpsimd.memset(xqa[:, 1:RA, WP - 1], 0.0)
            nc.gpsimd.memset(xqb[:, RB - 1, :], 0.0)
            nc.gpsimd.memset(xqb[:, 0 : RB - 1, 0], 0.0)
            nc.gpsimd.memset(xqb[:, 0 : RB - 1, WP - 1], 0.0)

            if im in xr_tiles:
                xr = xr_tiles[im]
            else:
                xr = xraw.tile([C, H * W], f32, name="xr")
                nc.sync.dma_start(xr[:], xs.ap()[im].rearrange("c h w -> c (h w)"))

            # band A: orig rows 0..32 -> xqa rows 1..33
            # band B: orig rows 31..55 -> xqb rows 0..24
            bands = [
                (0, RA - 1, xqa, 1),   # (orig row start, n rows, tile, tile row offset)
                (POB - 1, H - POB + 1, xqb, 0),
            ]
            for bi, (r0, nr, xqt, toff) in enumerate(bands):
                sl = slice(r0 * W, (r0 + nr) * W)
                t1 = tq.tile([C, H * W], f32, name=f"t1_{bi}")
                nc.vector.tensor_scalar(
                    t1[:, 0 : nr * W], xr[:, sl], QA, 0.0, op0=ALU.mult, op1=ALU.max
                )
                t2 = tq.tile([C, H * W], f32, name=f"t2_{bi}")
                nc.vector.tensor_scalar(
                    t2[:, 0 : nr * W], t1[:, 0 : nr * W], 255.0, MAGIC,
                    op0=ALU.min, op1=ALU.add,
                )
                nc.vector.tensor_scalar(
                    xqt[:, toff : toff + nr, 1 : W + 1],
                    t2[:, 0 : nr * W].rearrange("c (h w) -> c h w", w=W),
                    MAGIC, None, op0=ALU.subtract,
                )

            # stationary reuse: per (half, band, rs) load the 128x128 weight
            # once and sweep every chunk of the band under it (144 loads
            # total instead of 504 -- LDWEIGHTS may not fully hide behind
            # 448-col matmuls on silicon)
            for half in range(2):
                for grp in (range(0, CHA), range(CHA, NCHUNK)):
                    # final band of the kernel: per-chunk sweep so the PSUM
                    # drains stagger instead of bursting after the last matmul;
                    # the very last chunk is split into two 4-row subtiles so
                    # its epilogue/DMA overlaps the closing matmuls
                    stagger = im == NIMG - 1 and half == 1 and grp.start == CHA
                    # work units: (chunk, sub, row offset within chunk, n rows)
                    units = []
                    for ch in grp:
                        if stagger and ch == NCHUNK - 1:
                            hrows = ROWS_PER_CHUNK // 2
                            units.append((ch, 0, 0, hrows))
                            units.append((ch, 1, hrows, ROWS_PER_CHUNK - hrows))
                        else:
                            units.append((ch, 0, 0, ROWS_PER_CHUNK))
                    ps_tiles = {}
                    for ch, sub, ro, nrw in units:
                        ps_tiles[(ch, sub)] = psum.tile(
                            [128, nrw * W], f32, name="ps", bufs=6
                        )
                    if stagger:
                        order = [
                            (u, r, s)
                            for u in units
                            for r in range(3)
                            for s in range(3)
                        ]
                    else:
                        order = [
                            (u, r, s)
                            for r in range(3)
                            for s in range(3)
                            for u in units
                        ]
                    for (ch, sub, ro, nrw), r, s in order:
                        rs = r * 3 + s
                        if ch < CHA:
                            xq_t, rbase = xqa, ch * ROWS_PER_CHUNK
                        else:
                            xq_t, rbase = xqb, ch * ROWS_PER_CHUNK - POB
                        rb = rbase + ro
                        nc.tensor.matmul(
                            ps_tiles[(ch, sub)][:],
                            wqh[half][:, rs, :],
                            xq_t[:, rb + r : rb + r + nrw, s : s + W],
                            start=(rs == 0),
                            stop=(rs == 8),
                        )
                    for ch, sub, ro, nrw in units:
                        ps = ps_tiles[(ch, sub)]
                        nf = nrw * W
                        tpost = post.tile([128, FREE], f32, name="tpost")
                        nc.scalar.activation(
                            tpost[:, 0:nf], ps[:], ACTF.Relu,
                            bias=b2[:, half : half + 1], scale=a2[:, half : half + 1],
                        )
                        u = post.tile([128, FREE], f32, name="u")
                        nc.vector.tensor_scalar(
                            u[:, 0:nf], tpost[:, 0:nf], 255.0, MAGIC,
                            op0=ALU.min, op1=ALU.add,
                        )
                        ov = outp.tile([128, FREE], f32, name="ov")
                        nc.vector.tensor_scalar(
                            ov[:, 0:nf], u[:, 0:nf], MAGIC, STEP,
                            op0=ALU.subtract, op1=ALU.mult,
                        )
                        nc.sync.dma_start(
                            out.ap()[im, half * 128 : (half + 1) * 128]
                            .rearrange("o h w -> o (h w)")[
                                :, ch * FREE + ro * W : ch * FREE + ro * W + nf
                            ],
                            ov[:, 0:nf],
                        )


_CACHED = None


def _get_program():
    global _CACHED
    if _CACHED is None:
        nc = bacc.Bacc(
            "TRN2", target_bir_lowering=False, debug=False, num_devices=NCORES
        )
        with tile.TileContext(nc) as tc:
            _build_body(tc)
        nc.compile()
        _CACHED = nc
    return _CACHED


def run_on_cores(inputs, trace=False, **kw):
    """Run the SPMD kernel; returns (full_output, BassKernelResults)."""
    nc = _get_program()
    x = np.ascontiguousarray(inputs["x"], dtype=np.float32)
    in_maps = []
    for c in range(NCORES):
        in_maps.append(
            {
                "xs": np.ascontiguousarray(x[c * NIMG : (c + 1) * NIMG]),
                "wt": np.ascontiguousarray(inputs["weight"], dtype=np.float32),
                "gm": np.ascontiguousarray(inputs["gamma"], dtype=np.float32),
                "bt": np.ascontiguousarray(inputs["beta"], dtype=np.float32),
                "mn": np.ascontiguousarray(inputs["mean"], dtype=np.float32),
                "vr": np.ascontiguousarray(inputs["var"], dtype=np.float32),
            }
        )
    res = run_bass_kernel_spmd(nc, in_maps, list(range(NCORES)), trace=trace, **kw)
    full = np.concatenate([res.results[c]["out"] for c in range(NCORES)], axis=0)
    return full.astype(np.float32), res


def kernel(**inputs) -> np.ndarray:
    full, _ = run_on_cores(inputs)
    return full


# revision 33
# speedup vs baseline: 57897.5138x; 1.0036x over previous
"""Trainium2 Bass kernel for quantized ConvBlock (fake-quant -> conv3x3 -> BN -> relu6 fake-quant).

Strategy
--------
Data-parallel over batch: 32 images -> 4 per NeuronCore x 8 cores.

Math: the reference fake-quantizes activations to the 256-level grid
k*(6/255), k in [0,255], and weights to m*(s/127), m in [-127,127],
s = max|w|.  Both integer grids are exactly representable in bf16, so the
conv reduces to an *integer* matmul accumulated in fp32 PSUM — exact —
and runs at full bf16 TensorE rate.  Per (r,s) tap the 3x3 conv is a
128(Cin) x 128(Cout-half) matmul over pixels; 9 taps accumulate in PSUM.

Rounding: ACT has no rint, so round-to-nearest-even is done on DVE with
the fp32 magic-number trick (v + 1.5*2^23) - 1.5*2^23, valid for
|v| < 2^22 (values here are <= ~1e5).

Fused epilogue per PSUM tile:
  ACT : t = Relu(a2_c * conv + b2_c)     (a2 = 42.5*bn_scale*qscale, per-channel)
  DVE : u = min(t,255) + MAGIC
  DVE : out = (u - MAGIC) * (6/255)
"""

import numpy as np

import concourse.bass as bass
import concourse.mybir as mybir
import concourse.tile as tile
from concourse import bacc, bass_isa
from concourse.bass_utils import run_bass_kernel_spmd

# ---- problem constants (hardcoded per contract) ----
N, C, H, W = 32, 128, 56, 56
O = 256
NCORES = 8
NIMG = N // NCORES  # images per core
HP, WP = H + 2, W + 2  # zero-padded input plane
ROWS_PER_CHUNK = 8
NCHUNK = H // ROWS_PER_CHUNK  # 7
FREE = ROWS_PER_CHUNK * W  # 448 <= 512 (one PSUM bank)

MAGIC = 12582912.0  # 1.5 * 2**23 : fp32 RNE round-to-int trick
QA = 42.5  # 255/6
STEP = float(np.float32(6.0 / 255.0))
BN_EPS = 1e-5

f32 = mybir.dt.float32
bf16 = mybir.dt.bfloat16
ALU = mybir.AluOpType
ACTF = mybir.ActivationFunctionType


def _build_body(tc):
    nc = tc.nc
    xs = nc.dram_tensor("xs", [NIMG, C, H, W], f32, kind="ExternalInput")
    wt = nc.dram_tensor("wt", [O, C, 3, 3], f32, kind="ExternalInput")
    gm = nc.dram_tensor("gm", [O], f32, kind="ExternalInput")
    bt = nc.dram_tensor("bt", [O], f32, kind="ExternalInput")
    mn = nc.dram_tensor("mn", [O], f32, kind="ExternalInput")
    vr = nc.dram_tensor("vr", [O], f32, kind="ExternalInput")
    out = nc.dram_tensor("out", [NIMG, O, H, W], f32, kind="ExternalOutput")

    from contextlib import ExitStack

    with ExitStack() as ctx:
        const = ctx.enter_context(tc.tile_pool(name="const", bufs=1))
        wpool = ctx.enter_context(tc.tile_pool(name="wpool", bufs=1))
        xqpool = ctx.enter_context(tc.tile_pool(name="xqp", bufs=1))
        xraw = ctx.enter_context(tc.tile_pool(name="xraw", bufs=2))
        tq = ctx.enter_context(tc.tile_pool(name="tq", bufs=2))
        psum = ctx.enter_context(tc.tile_pool(name="psum", bufs=8, space="PSUM"))
        post = ctx.enter_context(tc.tile_pool(name="post", bufs=4))
        outp = ctx.enter_context(tc.tile_pool(name="outp", bufs=4))

        from concourse.masks import make_identity

        ident = const.tile([128, 128], bf16)
        make_identity(nc, ident[:])

        # ================= weights: load, quantize to integer bf16 =========
        # Natural-layout contiguous load [o(part), i*9] (2 halves), quantize
        # there, then 18 PE transposes (bf16) into stationary layout
        # [i(part), rs, o] per half.  Avoids the 36B-descriptor strided gather.
        wt_nat = wt.ap().rearrange("o i h w -> o (i h w)")
        wnat = []
        for h in range(2):
            wn = wpool.tile([128, C * 9], f32, name=f"wnat{h}")
            # one half per HWDGE ring (SP / ACT): parallel triggers on silicon
            eng = nc.sync if h == 0 else nc.scalar
            eng.dma_start(wn[:], wt_nat[h * 128 : (h + 1) * 128, :])
            wnat.append(wn)

        # ========== BN constants, part 1 (independent of weight scale) ======
        # Emitted first so the single ACT Sqrt triggers the (only) activation
        # table-set load at t~0; Relu is a filler fn present in every set.
        # channel (h*128+p) -> partition p, column h
        gmt = const.tile([128, 2], f32)
        btt = const.tile([128, 2], f32)
        mnt = const.tile([128, 2], f32)
        vrt = const.tile([128, 2], f32)
        nc.sync.dma_start(vrt[:], vr.ap().rearrange("(h p) -> p h", p=128))
        nc.sync.dma_start(gmt[:], gm.ap().rearrange("(h p) -> p h", p=128))
        nc.sync.dma_start(btt[:], bt.ap().rearrange("(h p) -> p h", p=128))
        nc.sync.dma_start(mnt[:], mn.ap().rearrange("(h p) -> p h", p=128))

        # bnscale = gamma * rsqrt(var+eps): ACT Sqrt seed + approx-recip +
        # 2 Newton rsqrt refinements (all cheap DVE ops, no iterative divide)
        veps = const.tile([128, 2], f32)
        nc.vector.tensor_scalar(veps[:], vrt[:], BN_EPS, None, op0=ALU.add)
        sv = const.tile([128, 2], f32)
        nc.scalar.activation(sv[:], veps[:], ACTF.Sqrt)
        r_scr = const.tile([128, 2], f32)
        r_cur = const.tile([128, 2], f32)
        nc.vector.reciprocal_approx_accurate(r_cur[:], sv[:], r_scr[:])
        cur = r_cur
        for it in range(2):
            t_sq = const.tile([128, 2], f32, name=f"rs_t{it}")
            nc.vector.tensor_tensor(t_sq[:], cur[:], cur[:], op=ALU.mult)
            t_u = const.tile([128, 2], f32, name=f"rs_u{it}")
            nc.vector.tensor_tensor(t_u[:], veps[:], t_sq[:], op=ALU.mult)
            t_c = const.tile([128, 2], f32, name=f"rs_c{it}")
            nc.vector.tensor_scalar(t_c[:], t_u[:], -0.5, 1.5, op0=ALU.mult, op1=ALU.add)
            t_n = const.tile([128, 2], f32, name=f"rs_n{it}")
            nc.vector.tensor_tensor(t_n[:], cur[:], t_c[:], op=ALU.mult)
            cur = t_n
        bnscale = const.tile([128, 2], f32)
        nc.vector.tensor_tensor(bnscale[:], gmt[:], cur[:], op=ALU.mult)
        # b2 = 42.5 * (beta - mean*bnscale)
        msc = const.tile([128, 2], f32)
        nc.vector.tensor_tensor(msc[:], mnt[:], bnscale[:], op=ALU.mult)
        bmm = const.tile([128, 2], f32)
        nc.vector.tensor_tensor(bmm[:], btt[:], msc[:], op=ALU.subtract)
        b2 = const.tile([128, 2], f32)
        nc.vector.tensor_scalar(b2[:], bmm[:], QA, None, op0=ALU.mult)


        # x image 0 load right after weights: its quant chain overlaps weight prep
        xr_tiles = {}
        xr0 = xraw.tile([C, H * W], f32, name="xr")
        nc.sync.dma_start(xr0[:], xs.ap()[0].rearrange("c h w -> c (h w)"))
        xr_tiles[0] = xr0


        habs = []
        for h in range(2):
            t = const.tile([128, 1], f32, name=f"wabs{h}")
            nc.vector.tensor_reduce(
                t[:], wnat[h][:], axis=mybir.AxisListType.X, op=ALU.max,
                apply_absolute_value=True,
            )
            habs.append(t)
        wabs = const.tile([128, 1], f32)
        nc.vector.tensor_tensor(wabs[:], habs[0][:], habs[1][:], op=ALU.max)
        smax = const.tile([C, 1], f32)
        nc.gpsimd.partition_all_reduce(
            smax[:], wabs[:], channels=C, reduce_op=bass_isa.ReduceOp.absmax
        )
        # 1/s via approx reciprocal + extra Newton pass (sub-ULP, ~2x faster
        # than the iterative-divide reciprocal which sits on the critical path)
        from concourse.dve_ops import RECIPROCAL_APPROX_NR

        rscr = const.tile([C, 1], f32)
        srcp = const.tile([C, 1], f32)
        nc.vector.reciprocal_approx_accurate(srcp[:], smax[:], rscr[:])
        srcp2 = const.tile([C, 1], f32)
        nc.vector._custom_dve(
            RECIPROCAL_APPROX_NR, out=srcp2[:], in0=smax[:], in1=srcp[:], s0=2.0
        )
        winv = const.tile([C, 1], f32)  # 127/s
        nc.vector.tensor_scalar(winv[:], srcp2[:], 127.0, None, op0=ALU.mult)

        # quantize in natural layout: integers in [-127,127], bf16
        wqn = []
        for h in range(2):
            wtmp = wpool.tile([128, C * 9], f32, name=f"wtmp{h}")
            nc.vector.tensor_scalar(
                wtmp[:], wnat[h][:], winv[:], MAGIC, op0=ALU.mult, op1=ALU.add
            )
            wq_h = wpool.tile([128, C * 9], bf16, name=f"wqn{h}")
            nc.vector.tensor_scalar(wq_h[:], wtmp[:], MAGIC, None, op0=ALU.subtract)
            wqn.append(wq_h)

        # a2 = bnscale * s/127   (42.5 * 6/255 == 1)
        qs2 = const.tile([128, 1], f32)
        nc.vector.tensor_scalar(qs2[:], smax[:], 1.0 / 127.0, None, op0=ALU.mult)
        a2 = const.tile([128, 2], f32)
        nc.vector.tensor_scalar(a2[:], bnscale[:], qs2[:], None, op0=ALU.mult)

        # transpose [o, i] -> [i, o] per (half, rs) via PE; per-half wq tiles
        # so half-0 matmuls unblock before half-1 transposes finish
        wqh = []
        for h in range(2):
            wq_t = wpool.tile([C, 9, 128], bf16, name=f"wqT{h}")
            wqn_r = wqn[h][:].rearrange("o (i r) -> o r i", r=9)
            for rs in range(9):
                pst = psum.tile([128, 128], bf16, name="pst", bufs=2)
                nc.tensor.transpose(pst[:], wqn_r[:, rs, :], ident[:])
                if rs % 2 == 0:
                    nc.scalar.copy(wq_t[:, rs, :], pst[:])
                else:
                    nc.vector.tensor_copy(wq_t[:, rs, :], pst[:])
            wqh.append(wq_t)

        # ================= per-image: quantize input, conv, epilogue =======
        # padded input plane split into two row-bands with a 2-row overlap so
        # the first chunks' matmuls start before the whole image is quantized:
        #   band A = padded rows 0..33  (chunks 0..3), writes orig rows 0..32
        #   band B = padded rows 32..57 (chunks 4..6), writes orig rows 31..55
        CHA = 4  # chunks in band A
        RA = CHA * ROWS_PER_CHUNK + 2  # 34 padded rows
        POB = (CHA * ROWS_PER_CHUNK) - 2  # band B starts at padded row 32
        RB = HP - POB  # 26 padded rows

        for im in range(NIMG):
            xqa = xqpool.tile([C, RA, WP], bf16, name=f"xqa{im}")
            xqb = xqpool.tile([C, RB, WP], bf16, name=f"xqb{im}")
            # zero only the pad ring; interior is overwritten by the quant pass
            nc.gpsimd.memset(xqa[:, 0, :], 0.0)
            nc.gpsimd.memset(xqa[:, 1:RA, 0], 0.0)
            nc.gpsimd.memset(xqa[:, 1:RA, WP - 1], 0.0)
            nc.gpsimd.memset(xqb[:, RB - 1, :], 0.0)
            nc.gpsimd.memset(xqb[:, 0 : RB - 1, 0], 0.0)
            nc.gpsimd.memset(xqb[:, 0 : RB - 1, WP - 1], 0.0)

            if im in xr_tiles:
                xr = xr_tiles[im]
            else:
                xr = xraw.tile([C, H * W], f32, name="xr")
                nc.sync.dma_start(xr[:], xs.ap()[im].rearrange("c h w -> c (h w)"))

            # band A: orig rows 0..32 -> xqa rows 1..33
            # band B: orig rows 31..55 -> xqb rows 0..24
            bands = [
                (0, RA - 1, xqa, 1),   # (orig row start, n rows, tile, tile row offset)
                (POB - 1, H - POB + 1, xqb, 0),
            ]
            for bi, (r0, nr, xqt, toff) in enumerate(bands):
                sl = slice(r0 * W, (r0 + nr) * W)
                t1 = tq.tile([C, H * W], f32, name=f"t1_{bi}")
                nc.vector.tensor_scalar(
                    t1[:, 0 : nr * W], xr[:, sl], QA, 0.0, op0=ALU.mult, op1=ALU.max
                )
                t2 = tq.tile([C, H * W], f32, name=f"t2_{bi}")
                nc.vector.tensor_scalar(
                    t2[:, 0 : nr * W], t1[:, 0 : nr * W], 255.0, MAGIC,
                    op0=ALU.min, op1=ALU.add,
                )
                nc.vector.tensor_scalar(
                    xqt[:, toff : toff + nr, 1 : W + 1],
                    t2[:, 0 : nr * W].rearrange("c (h w) -> c h w", w=W),
                    MAGIC, None, op0=ALU.subtract,
                )

            # stationary reuse: per (half, band, rs) load the 128x128 weight
            # once and sweep every chunk of the band under it (144 loads
            # total instead of 504 -- LDWEIGHTS may not fully hide behind
            # 448-col matmuls on silicon)
            for half in range(2):
                for grp in (range(0, CHA), range(CHA, NCHUNK)):
                    # final band of the kernel: per-chunk sweep so the PSUM
                    # drains stagger instead of bursting after the last matmul;
                    # the very last chunk is split into two 4-row subtiles so
                    # its epilogue/DMA overlaps the closing matmuls
                    stagger = im == NIMG - 1 and half == 1 and grp.start == CHA
                    # work units: (chunk, sub, row offset within chunk, n rows)
                    units = []
                    for ch in grp:
                        if stagger and ch == NCHUNK - 1:
                            hrows = ROWS_PER_CHUNK // 2
                            units.append((ch, 0, 0, hrows))
                            units.append((ch, 1, hrows, ROWS_PER_CHUNK - hrows))
                        else:
                            units.append((ch, 0, 0, ROWS_PER_CHUNK))
                    ps_tiles = {}
                    for ch, sub, ro, nrw in units:
                        ps_tiles[(ch, sub)] = psum.tile(
                            [128, nrw * W], f32, name="ps", bufs=6
                        )
                    if stagger:
                        order = [
                            (u, r, s)
                            for u in units
                            for r in range(3)
                            for s in range(3)
                        ]
                    else:
                        order = [
                            (u, r, s)
                            for r in range(3)
                            for s in range(3)
                            for u in units
                        ]
                    for (ch, sub, ro, nrw), r, s in order:
                        rs = r * 3 + s
                        if ch < CHA:
                            xq_t, rbase = xqa, ch * ROWS_PER_CHUNK
                        else:
                            xq_t, rbase = xqb, ch * ROWS_PER_CHUNK - POB
                        rb = rbase + ro
                        nc.tensor.matmul(
                            ps_tiles[(ch, sub)][:],
                            wqh[half][:, rs, :],
                            xq_t[:, rb + r : rb + r + nrw, s : s + W],
                            start=(rs == 0),
                            stop=(rs == 8),
                        )
                    for ch, sub, ro, nrw in units:
                        ps = ps_tiles[(ch, sub)]
                        nf = nrw * W
                        tpost = post.tile([128, FREE], f32, name="tpost")
                        nc.scalar.activation(
                            tpost[:, 0:nf], ps[:], ACTF.Relu,
                            bias=b2[:, half : half + 1], scale=a2[:, half : half + 1],
                        )
                        u = post.tile([128, FREE], f32, name="u")
                        nc.vector.tensor_scalar(
                            u[:, 0:nf], tpost[:, 0:nf], 255.0, MAGIC,
                            op0=ALU.min, op1=ALU.add,
                        )
                        ov = outp.tile([128, FREE], f32, name="ov")
                        nc.vector.tensor_scalar(
                            ov[:, 0:nf], u[:, 0:nf], MAGIC, STEP,
                            op0=ALU.subtract, op1=ALU.mult,
                        )
                        nc.sync.dma_start(
                            out.ap()[im, half * 128 : (half + 1) * 128]
                            .rearrange("o h w -> o (h w)")[
                                :, ch * FREE + ro * W : ch * FREE + ro * W + nf
                            ],
                            ov[:, 0:nf],
                        )


_CACHED = None


def _get_program():
    global _CACHED
    if _CACHED is None:
        nc = bacc.Bacc(
            "TRN2", target_bir_lowering=False, debug=False, num_devices=NCORES
        )
        with tile.TileContext(nc) as tc:
            _build_body(tc)
        nc.compile()
        _CACHED = nc
    return _CACHED


def run_on_cores(inputs, trace=False, **kw):
    """Run the SPMD kernel; returns (full_output, BassKernelResults)."""
    nc = _get_program()
    x = np.ascontiguousarray(inputs["x"], dtype=np.float32)
    in_maps = []
    for c in range(NCORES):
        in_maps.append(
            {
                "xs": np.ascontiguousarray(x[c * NIMG : (c + 1) * NIMG]),
                "wt": np.ascontiguousarray(inputs["weight"], dtype=np.float32),
                "gm": np.ascontiguousarray(inputs["gamma"], dtype=np.float32),
                "bt": np.ascontiguousarray(inputs["beta"], dtype=np.float32),
                "mn": np.ascontiguousarray(inputs["mean"], dtype=np.float32),
                "vr": np.ascontiguousarray(inputs["var"], dtype=np.float32),
            }
        )
    res = run_bass_kernel_spmd(nc, in_maps, list(range(NCORES)), trace=trace, **kw)
    full = np.concatenate([res.results[c]["out"] for c in range(NCORES)], axis=0)
    return full.astype(np.float32), res


def kernel(**inputs) -> np.ndarray:
    full, _ = run_on_cores(inputs)
    return full
